# revision 25
# baseline (speedup 1.0000x reference)
"""Masked multi-head attention (fused QKV) on 8 trn2 NeuronCores.

Problem (full shapes): x [2, 2048, 1024] f32, W [3072, 1024], b [3072].
  z = x @ W.T + b ; k,q,v = split(z) ; heads H=16, hd=64
  out = softmax(causal(q k^T / sqrt(1024))) v   -> [2, 2048, 1024]

Sharding: core c handles batch n=c//4 and head group g=c%4 (4 heads).
Each core is fully independent (data + head parallel, no collectives).

Per-core device program (v2 of the fp8-DoubleRow kernel):
  0) Projections run as residual-fp8 DoubleRow: x = x_hi + x_lo and
     16*W = W_hi + W_lo in e4m3 (host-side), z = xh@Wh + xh@Wl + xl@Wh
     accumulated in f32 PSUM.  Host packs hi/lo as one [D, 2, *] tensor
     per input so big blocks move in one DMA each.
  1) kq weights are hp-major ([k_hp0|q_hp0|k_hp1|q_hp1]) so the startup
     critical path only loads the hp0 half (256 cols) before the first
     projection; DMA order is cost-model-driven (the 16-engine DMA pool
     serializes transfers at ~360 B/ns and each completion semaphore
     takes ~900ns to propagate).
  2) Scores per (q-block 512, head pair) unit: 2 fp8 DoubleRow matmuls
     per k-tile into a 2-bank PSUM tile, one ACT exp (the global pacer)
     evacuates both heads to bf16 pt.  Unit order
     (0,0),(0,1),(1,0),(2,0),(3,0),(3,1),(2,1),(1,1) starts on the two
     smallest units (whose projections are resident earliest) and ends
     on a mid-size unit whose own PV chains drain history-first.
  3) Filler work (deferred projections, the previous unit's PV chains)
     is placed by a greedy scheduler that simulates ACT/PE prefix times
     with the instruction cost model and only inserts a filler where it
     cannot stall the exp stream; per-filler DMA-readiness estimates
     and deadline slots (for next unit's diag dependencies) constrain
     placement.
  4) PV in natural layout, f32 PSUM [128, 65] per (head, q-tile); DVE
     reciprocal of the fused denominator column + tensor_scalar_mul.
     Late units stream per-q-tile output DMAs so the tail is short.

Numerics identical to v1: bf16 projection ~2e-4; fp8 q/k quantization
dominates at rel_err ~1.3e-2 (harness gate 2e-2).

_split_matmul_waits() is a required legalization for this compiler
build: every engine instruction may carry at most one semaphore wait.
"""

import numpy as np

import concourse.bass as bass
import concourse.mybir as mybir
import concourse.tile as tile
from concourse.bass_utils import run_bass_kernel_spmd

F32 = mybir.dt.float32
BF = mybir.dt.bfloat16
F8 = mybir.dt.float8e4

FP8_SCORES = True

N, S, D = 2, 2048, 1024
H, HD = 16, 64
P = 128
QB = 512                 # q block (free dim per matmul)
NQB = S // QB            # 4
NKT = S // P             # 16 k tiles
ND = D // P              # 8 contraction tiles
NHC = 4                  # heads per core
EKQ = 2 * NHC * HD       # 512 = k+q rows per core
EV = NHC * HD            # 256 = v rows per core
WS = 16.0                # host W/b pre-scale (fp8 range)
SCALE = 1.0 / 32.0 / (WS * WS)   # 1/sqrt(1024), W-scale compensated

AF = mybir.ActivationFunctionType
ALU = mybir.AluOpType

# ---- cost-model constants for the greedy filler scheduler ----
ACT_C = 0.8333           # ns/elem on ACT
PE_C = 0.4167            # ns/col full rate
SEM = 160.0              # cross-engine handoff guard
GUARD = 120.0


def _exp_ns(w):
    return 2 * w * ACT_C + 185.0


def _mm_tile_ns(w, direct):
    # two matmuls (one per interleaved head) per k-tile
    if not FP8_SCORES:
        return 2 * w * PE_C
    return (2 * w * PE_C) if direct else (w * PE_C)


def _split_matmul_waits(nc):
    """Move extra semaphore waits onto preceding same-engine NOPs.

    The walrus codegen for self-loading matmuls folds waits into the
    LDWEIGHTS struct, which has room for a single sync-wait command;
    sequencer NOPs on the same engine execute in program order, so
    hoisting each wait onto its own NOP is semantics-preserving.
    """
    import bass_rust

    moved = 0
    for bb in nc.main_func.blocks:
        out = []
        for ins in bb.instructions:
            si = ins.sync_info
            keep = 0 if isinstance(ins, bass_rust.InstMatmult) else 1
            if (
                not isinstance(ins, bass_rust.InstNoOp)
                and si is not None
                and len(si.on_wait) > keep
            ):
                hoist = si.on_wait[keep:] if keep else si.on_wait
                for j, w in enumerate(hoist):
                    out.append(
                        bass_rust.InstNoOp(
                            name=f"{ins.name}-hw{j}",
                            engine=ins.engine,
                            sync_info=mybir.SyncInfo(on_wait=[w], on_update=[]),
                        )
                    )
                    moved += 1
                ins.sync_info = mybir.SyncInfo(
                    on_wait=list(si.on_wait[:keep]), on_update=list(si.on_update)
                )
            out.append(ins)
        bb.instructions[:] = out
    return moved


def build_nc(split_waits=True):
    nc = bass.Bass()

    # Contraction rows are host-permuted p-major (row r = p*ND + dt) so
    # every DMA slice below keeps >=512B contiguous runs (the DMA cost
    # model halves bandwidth under 512B).  x and wv pack hi/lo planes
    # with u inner; wkq is split per (head pair, plane) so the startup
    # loads only what the first projection needs.
    x2 = nc.dram_tensor("x2", [2 * D, S], F8, kind="ExternalInput")
    wkqt_d = [[nc.dram_tensor(f"wkq{hp}{u}", [D, 2 * P], F8,
                              kind="ExternalInput")
               for u in range(2)] for hp in range(2)]
    wv_d = [nc.dram_tensor(f"wv{u}", [D, EV], F8, kind="ExternalInput")
            for u in range(2)]
    bkq = nc.dram_tensor("bkq", [P, 4], F32, kind="ExternalInput")
    bv = nc.dram_tensor("bv", [1, EV], F32, kind="ExternalInput")
    o = nc.dram_tensor("o", [S, EV], F32, kind="ExternalOutput")

    x_v = x2.rearrange("(p dt u) s -> p dt u s", p=P, u=2)  # [128,8,2,2048]
    wkq_vs = [[wkqt_d[hp][u].rearrange("(p dt) e -> p dt e", p=P)
               for u in range(2)] for hp in range(2)]       # [128,8,256]
    wv_vs = [wv_d[u].rearrange("(p dt) e -> p dt e", p=P)
             for u in range(2)]                             # [128,8,256]
    o_v = o.rearrange("(qt p) c -> p qt c", p=P)            # [128, 16, 256]

    with tile.TileContext(nc) as tc:
        with (
            tc.tile_pool(name="const", bufs=1) as const,
            tc.tile_pool(name="big", bufs=1) as big,
            tc.tile_pool(name="xpool", bufs=2) as xpool,
            tc.tile_pool(name="work", bufs=2) as work,
            tc.tile_pool(name="opool", bufs=2) as opool,
            tc.tile_pool(name="proj_ps", bufs=2, space="PSUM") as proj_ps,
            tc.tile_pool(name="st_ps", bufs=2, space="PSUM") as st_ps,
            tc.tile_pool(name="pv_ps", bufs=2, space="PSUM") as pv_ps,
        ):
            # ---- constants ----
            onesb = const.tile([P, 1], BF)
            nc.vector.memset(onesb, 1.0)
            wsb = const.tile([P, 1], BF)
            nc.vector.memset(wsb, WS)
            # warm the ACT exp table while DMAs run
            dummy = const.tile([1, 2], F32)
            nc.gpsimd.memset(dummy, 0.0)
            nc.scalar.activation(dummy, dummy, AF.Exp)
            # diagonal causal mask for the odd head: mask[p, j] = (j >= p)
            mask_sb = const.tile([P, QB], BF)
            nc.gpsimd.affine_select(
                out=mask_sb,
                in_=onesb.to_broadcast((P, QB)),
                compare_op=ALU.is_ge, fill=0.0,
                base=0, channel_multiplier=-1,
                pattern=[[1, QB]],
            )

            # ---- input DMAs, critical-path first ----
            wkqt = [[const.tile([P, ND, 2 * P], F8, name=f"wkq{hp}{u}")
                     for u in range(2)] for hp in range(2)]
            wvt = [const.tile([P, ND, EV], F8, name=f"wv{u}")
                   for u in range(2)]
            bkq_sb = const.tile([P, 4], F32)
            bvb = const.tile([P, EV], F32)
            xqbs = []
            for qb in range(NQB):
                xq = xpool.tile([P, ND, 2, QB], F8, tag=f"xqb{qb}", bufs=1,
                                name=f"xqb{qb}")
                xqbs.append(xq)

            # DMA cost model: gens serialize on HWDGE (625ns each),
            # transfers serialize on the 16-engine pool at ~360 B/ns,
            # completion semaphores take ~950ns to reach consumers.
            st_dma = {"gen": 1.06e3, "tx": 0.0}

            def dma(out_ap, in_ap, kbytes):
                nc.sync.dma_start(out_ap, in_ap)
                st_dma["gen"] += 625.0
                start = max(st_dma["gen"], st_dma["tx"])
                st_dma["tx"] = start + kbytes * 1024.0 / 360.0
                return st_dma["tx"] + 950.0

            r_bkq = dma(bkq_sb, bkq[:, :], 2)
            r_wkq00 = dma(wkqt[0][0], wkq_vs[0][0], 256)
            r_xq0h = dma(xqbs[0][:, :, 0], x_v[:, :, 0, 0:QB], 512)
            r_xq0l = dma(xqbs[0][:, :, 1], x_v[:, :, 1, 0:QB], 512)
            r_wkq01 = dma(wkqt[0][1], wkq_vs[0][1], 256)
            r_xq1h = dma(xqbs[1][:, :, 0], x_v[:, :, 0, QB:2 * QB], 512)
            r_wkq10 = dma(wkqt[1][0], wkq_vs[1][0], 256)
            r_wkq11 = dma(wkqt[1][1], wkq_vs[1][1], 256)
            r_wvh = dma(wvt[0], wv_vs[0], 256)
            r_xq1l = dma(xqbs[1][:, :, 1], x_v[:, :, 1, QB:2 * QB], 512)
            r_wvl = dma(wvt[1], wv_vs[1], 256)
            r_bv = dma(bvb, bv[:, :].partition_broadcast(P), 1)
            r_xq2 = dma(xqbs[2][:, :, :], x_v[:, :, :, 2 * QB:3 * QB], 1024)
            r_xq3 = dma(xqbs[3][:, :, :], x_v[:, :, :, 3 * QB:4 * QB], 1024)
            x_ready = [(r_xq0h, r_xq0l), (r_xq1h, r_xq1l),
                       (r_xq2, r_xq2), (r_xq3, r_xq3)]
            wkq_ready = [(r_wkq00, r_wkq01), (r_wkq10, r_wkq11)]
            r_wv = max(r_wvl, r_bv)

            # ---- persistent state ----
            # zkq [p, kq, hp, s]; e-tile t -> (kq=t%2, hp=t//2)
            if FP8_SCORES:
                zkq = big.tile([P, 2, 2, S], F8)
                # DoubleRow re-layout [hl*32+p, kq, hp, g, s], hd = g*32+p
                kq8 = big.tile([HD, 2, 2, 2, S], F8)
            else:
                zkq = big.tile([P, 4, S], BF)
            # v natural + WS column: [p, ktile, head, 65]
            vsb = big.tile([P, NKT, NHC, HD + 1], BF)
            nc.vector.tensor_copy(
                vsb[:, :, :, HD:HD + 1],
                wsb[:, :, None].to_broadcast((P, NKT, NHC, 1)),
            )
            # exp(S^T) per unit: [p, head, ktile, q], double-buffered
            pta = big.tile([P, 2, NKT, QB], BF)
            ptb = big.tile([P, 2, NKT, QB], BF)
            pts = [pta, ptb]

            proj_state = {}
            DR = mybir.MatmulPerfMode.DoubleRow
            # residual passes: z = xh@Wh + xh@Wl + xl@Wh, in an order
            # chosen per granule so the last pass waits on the DMA that
            # lands last for that block.
            PASS_WL = ((0, 0), (1, 0), (0, 1))   # W-lo last (qb0 blocks)
            PASS_XL = ((0, 0), (0, 1), (1, 0))   # x-lo last (qb>=1)

            def proj_kq_half(qb, t, half, passes=PASS_XL, evac_act=False):
                if half == 0:
                    pzp = proj_ps.tile([P, QB], F32, tag="projps",
                                       name=f"pzp{qb}_{t}")
                    proj_state[(qb, t)] = pzp
                else:
                    pzp = proj_state.pop((qb, t)) if half == 2 \
                        else proj_state[(qb, t)]
                xi, wi = passes[half]
                for p2 in range(4):
                    nc.tensor.matmul(
                        pzp,
                        lhsT=wkqt[t // 2][wi][:, 2 * p2:2 * p2 + 2,
                                              (t % 2) * P:(t % 2 + 1) * P],
                        rhs=xqbs[qb][:, 2 * p2:2 * p2 + 2, xi, :],
                        start=(half == 0 and p2 == 0),
                        stop=(half == 2 and p2 == 3),
                        perf_mode=DR,
                    )
                if half < 2:
                    return
                if FP8_SCORES:
                    out = zkq[:, t % 2, t // 2, qb * QB:(qb + 1) * QB]
                else:
                    out = zkq[:, t, qb * QB:(qb + 1) * QB]
                if evac_act:
                    nc.scalar.activation(
                        out, pzp, AF.Identity, bias=bkq_sb[:, t:t + 1]
                    )
                else:
                    nc.vector.tensor_scalar_add(out, pzp, bkq_sb[:, t:t + 1])
                if FP8_SCORES and t % 2 == 1:
                    # both e-tiles of head pair hp = t//2 evacuated:
                    # DoubleRow re-layout via SBUF->SBUF DMAs
                    hp = t // 2
                    qw = slice(qb * QB, (qb + 1) * QB)
                    for hl in range(2):
                        for g in range(2):
                            nc.sync.dma_start(
                                kq8[hl * 32:(hl + 1) * 32, :, hp, g, qw],
                                zkq[hl * HD + g * 32:hl * HD + g * 32 + 32,
                                    :, hp, qw],
                            )

            def proj_v1(qb, qt4):
                qt = qb * 4 + qt4
                pvp = proj_ps.tile([P, QB], F32, tag="projps",
                                   name=f"pvp{qt}")
                for hi in range(3):
                    xi, wi = PASS_XL[hi]
                    for p2 in range(4):
                        nc.tensor.matmul(
                            pvp[:, :EV],
                            lhsT=xqbs[qb][:, 2 * p2:2 * p2 + 2, xi,
                                          qt4 * P:(qt4 + 1) * P],
                            rhs=wvt[wi][:, 2 * p2:2 * p2 + 2, :],
                            start=(hi == 0 and p2 == 0),
                            stop=(hi == 2 and p2 == 3),
                            perf_mode=DR,
                        )
                nc.vector.tensor_tensor(
                    vsb[:, qt, :, 0:HD],
                    pvp[:, :EV].rearrange("p (h d) -> p h d", d=HD),
                    bvb.rearrange("p (h d) -> p h d", d=HD),
                    mybir.AluOpType.add,
                )

            def pv_chunk(qb, hp, qt4, hl, osb, pt, hist_first=False,
                         dma_qt=False):
                nkt_q = 4 * qb + qt4 + 1
                if hist_first:
                    kts = list(range(nkt_q))
                else:
                    kts = list(range(4 * qb, nkt_q)) + list(range(0, 4 * qb))
                pvo = pv_ps.tile([P, HD + 1], F32, tag="pv", name="pvo")
                for i, kt in enumerate(kts):
                    nc.tensor.matmul(
                        pvo,
                        lhsT=pt[:, hl, kt, qt4 * P:(qt4 + 1) * P],
                        rhs=vsb[:, kt, 2 * hp + hl, :],
                        start=(i == 0), stop=(i == nkt_q - 1),
                    )
                h = 2 * hp + hl
                rs = work.tile([P, 1], F32, tag="rs", bufs=4)
                nc.vector.reciprocal(rs, pvo[:, HD:HD + 1])
                nc.vector.tensor_scalar_mul(
                    osb[:, qt4, HD * h:HD * (h + 1)],
                    pvo[:, 0:HD], rs,
                )
                if dma_qt and hl == 1:
                    nc.sync.dma_start(
                        o_v[:, qb * 4 + qt4, hp * P:(hp + 1) * P],
                        osb[:, qt4, hp * P:(hp + 1) * P],
                    )

            # ---- greedy filler scheduler state ----
            # Fillers live in global FIFO queues and may spill across
            # unit boundaries; (ui, slot) deadlines encode the real
            # consumers (pt-ring WAR at slot 4 of unit ui+2, kq8
            # relayout latency before a diag reader, etc).
            sched = {"pe": 0.0, "act": 0.0}
            queues = []
            exp_tail = []

            def attn_scores(ui, qb, hp, pt, hist_first=False):
                if hist_first:
                    kts = list(range(0, 4 * qb + 4))
                else:
                    kts = list(range(4 * qb, 4 * qb + 4)) + \
                        list(range(4 * qb))

                def emit(f):
                    f["fn"]()
                    sched["pe"] = max(sched["pe"], f.get("ready", 0.0)) \
                        + f["pe"]

                for i, kt in enumerate(kts):
                    pos = (ui, i)
                    r = kt - 4 * qb
                    diag = 0 <= r < 4
                    off = P * r if diag else 0
                    w = QB - off
                    mmc = _mm_tile_ns(w, diag)
                    # forced fillers (deadline reached)
                    for g in queues:
                        while g and g[0].get("dl") and g[0]["dl"] <= pos:
                            emit(g.pop(0))
                    # opportunistic fillers while ACT has slack
                    progress = True
                    while progress:
                        progress = False
                        for g in queues:
                            if (g and g[0].get("es", (-1, -1)) <= pos
                                    and g[0].get("ready", 0.0) <= sched["pe"]
                                    and sched["pe"] + g[0]["pe"] + mmc
                                    + GUARD <= sched["act"]):
                                emit(g.pop(0))
                                progress = True
                    # st ring WAR: mm_i waits exp_{i-2}
                    if len(exp_tail) >= 2:
                        sched["pe"] = max(sched["pe"],
                                          exp_tail[-2] + 100.0)
                    stp = st_ps.tile([P, 2, QB], F32, tag="st")
                    for hl in range(2):
                        if FP8_SCORES and diag:
                            base = HD * hl
                            nc.tensor.matmul(
                                stp[:, hl, off:QB],
                                lhsT=zkq[base:base + HD, 0, hp,
                                         kt * P:(kt + 1) * P],
                                rhs=zkq[base:base + HD, 1, hp,
                                        qb * QB + off:(qb + 1) * QB],
                                start=True, stop=True,
                            )
                        elif FP8_SCORES:
                            nc.tensor.matmul(
                                stp[:, hl, off:QB],
                                lhsT=kq8[hl * 32:(hl + 1) * 32, 0, hp, :,
                                         kt * P:(kt + 1) * P],
                                rhs=kq8[hl * 32:(hl + 1) * 32, 1, hp, :,
                                        qb * QB + off:(qb + 1) * QB],
                                start=True, stop=True,
                                perf_mode=DR,
                            )
                        else:
                            base = HD * hl
                            nc.tensor.matmul(
                                stp[:, hl, off:QB],
                                lhsT=zkq[base:base + HD, 2 * hp,
                                         kt * P:(kt + 1) * P],
                                rhs=zkq[base:base + HD, 2 * hp + 1,
                                        qb * QB + off:(qb + 1) * QB],
                                start=True, stop=True,
                            )
                    sched["pe"] += mmc
                    est = max(sched["act"], sched["pe"] + SEM)
                    nc.scalar.activation(
                        pt[:, :, kt, off:QB],
                        stp[:, :, off:QB],
                        AF.Exp, scale=SCALE,
                    )
                    sched["act"] = est + _exp_ns(w)
                    exp_tail.append(sched["act"])
                    del exp_tail[:-2]
                    if diag:
                        nc.gpsimd.affine_select(
                            out=pt[:, 0, kt, off:QB],
                            in_=pt[:, 0, kt, off:QB],
                            compare_op=ALU.is_ge, fill=0.0,
                            base=0, channel_multiplier=-1,
                            pattern=[[1, w]],
                        )
                        nc.vector.tensor_mul(
                            out=pt[:, 1, kt, off:QB],
                            in0=pt[:, 1, kt, off:QB],
                            in1=mask_sb[:, 0:w],
                        )

            def drain_queues():
                for g in queues:
                    while g:
                        g.pop(0)["fn"]()

            # ---- filler group constructors ----
            def K(qb2, pair, dl=None):
                passes = PASS_WL if qb2 == 0 else PASS_XL
                tiles = (2 * pair, 2 * pair + 1)
                out = []
                for t in tiles:
                    for h in range(3):
                        xi, wi = passes[h]
                        ready = max(x_ready[qb2][xi], wkq_ready[pair][wi])
                        f = {
                            "fn": (lambda t=t, h=h, q=qb2, ps=passes:
                                   proj_kq_half(q, t, h, passes=ps)),
                            "pe": 427.0, "ready": ready,
                        }
                        if dl is not None:
                            f["dl"] = dl
                        out.append(f)
                return out

            def V(qb2):
                ready = max(x_ready[qb2][1], r_wv, r_bv)
                return [
                    {"fn": (lambda q4=q4, q=qb2: proj_v1(q, q4)),
                     "pe": 640.0, "ready": ready}
                    for q4 in range(4)
                ]

            def PV(pqb, php, ppt, v_qb=None, dma_qt=False, dl=None):
                # previous unit's PV chains; optionally interleave the
                # V-projection whose vsb tiles those chains read
                out = []
                vg = V(v_qb) if v_qb is not None else []
                for q4 in range(4):
                    if vg:
                        out.append(vg[q4])
                    for hl in range(2):
                        out.append({
                            "fn": (lambda q4=q4, hl=hl:
                                   pv_chunk(pqb, php, q4, hl, osbs[pqb],
                                            ppt, dma_qt=dma_qt)),
                            "pe": (4 * pqb + q4 + 1) * 27.0 + 100.0,
                        })
                if dl is not None:
                    for f in out:
                        f["dl"] = dl
                return out

            # ---- prologue: pass-major proj of (k,q) head pair 0, qb0 ----
            for half in range(3):
                for t in (0, 1):
                    proj_kq_half(0, t, half, passes=PASS_WL,
                                 evac_act=(t == 1))

            # ---- unit schedule ----
            units = [(0, 0), (0, 1), (1, 0), (2, 0),
                     (3, 0), (3, 1), (2, 1), (1, 1)]
            # deadline for PV-of-(ui-1), keyed by current ui: slot 4 of
            # unit ui+1 (pt-ring WAR with its history exps), except the
            # deep units where the next writer's diag already overlaps.
            PV_DL = {1: (2, 4), 2: (3, 4), 3: (4, 4),
                     4: (6, 0), 5: (7, 0), 6: None, 7: None}
            # K-group deadlines: diag readers consume zkq directly, so a
            # K group must land right before its reader unit's slot 0;
            # kq8 relayout only feeds distant history readers.
            K_PLAN = {
                0: [(0, 1, (1, 0))],
                1: [(1, 0, (2, 0))],
                2: [(2, 0, (3, 0))],
                3: [(3, 0, (4, 0)), (1, 1, (5, 6))],
                4: [(3, 1, (5, 0)), (2, 1, (5, 10))],
            }
            osbs = {}
            prev = None
            sched["pe"] = 8100.0
            sched["act"] = 8300.0
            for ui, (qb, hp) in enumerate(units):
                pt = pts[ui % 2]
                last = ui == len(units) - 1
                if qb not in osbs:
                    osbs[qb] = opool.tile([P, 4, EV], F32, tag="osb",
                                          bufs=4, name=f"osb{qb}")
                if prev is not None:
                    pqb, php, ppt = prev
                    # V(j) interleaves with the first PV chains reading it
                    v_qb = {1: 0, 3: 1, 4: 2, 5: 3}.get(ui)
                    queues.append(PV(pqb, php, ppt, v_qb=v_qb,
                                     dma_qt=(ui == 7), dl=PV_DL[ui]))
                for kqb, kpair, kdl in K_PLAN.get(ui, []):
                    queues.append(K(kqb, kpair, dl=kdl))
                if last:
                    own = []
                    for q4 in range(4):
                        for hl in range(2):
                            own.append({
                                "fn": (lambda q4=q4, hl=hl:
                                       pv_chunk(qb, hp, q4, hl, osbs[qb],
                                                pt, hist_first=True,
                                                dma_qt=True)),
                                "pe": (4 * qb + q4 + 1) * 27.0 + 100.0,
                                "es": (ui, 4 * qb + q4 + 1),
                            })
                    queues.append(own)
                attn_scores(ui, qb, hp, pt, hist_first=last)
                if prev is not None and ui != 7:
                    # previous unit's half-row is complete
                    nc.sync.dma_start(
                        o_v[:, pqb * 4:(pqb + 1) * 4,
                            php * P:(php + 1) * P],
                        osbs[pqb][:, :, php * P:(php + 1) * P],
                    )
                prev = (qb, hp, pt)
            drain_queues()

    if split_waits:
        _split_matmul_waits(nc)
    return nc


_nc_cache = None


def _get_nc():
    global _nc_cache
    if _nc_cache is None:
        _nc_cache = build_nc()
    return _nc_cache


def make_in_maps(x, W, b):
    import ml_dtypes

    f8 = ml_dtypes.float8_e4m3
    # p-major row permutation of the contraction dim: row r = p*ND + dt
    # picks original row dt*128 + p (x and W share it, so z is identical)
    perm = (np.arange(D).reshape(ND, P).T).reshape(-1)

    def hilo(a):
        hi = a.astype(f8)
        lo = (a - hi.astype(np.float32)).astype(f8)
        return hi, lo

    def hilo_pk(a):
        # rows (p dt) -> interleave planes u inner: rows (p dt u)
        hi, lo = hilo(a)
        return np.ascontiguousarray(
            np.stack([hi, lo], axis=1).reshape(2 * a.shape[0], a.shape[1])
        )

    x = np.asarray(x, dtype=np.float32)
    W = np.asarray(W, dtype=np.float32)
    b = np.asarray(b, dtype=np.float32)
    in_maps = []
    x2s = [hilo_pk(x[n].T[perm]) for n in range(N)]
    for c in range(8):
        n, g = divmod(c, 4)
        rk = slice(256 * g, 256 * g + 256)
        rq = slice(D + 256 * g, D + 256 * g + 256)
        rv = slice(2 * D + 256 * g, 2 * D + 256 * g + 256)
        Wk, Wq, Wv = W[rk], W[rq], W[rv]
        m = {"x2": x2s[n],
             "wv2": hilo_pk(Wv.T[perm] * 16.0),
             "bv": np.ascontiguousarray(b[rv].reshape(1, EV) * 16.0)}
        for hp in range(2):
            hr = slice(128 * hp, 128 * hp + 128)
            cols = np.concatenate([Wk[hr], Wq[hr]], axis=0).T[perm] * 16.0
            m[f"wkq{hp}0"], m[f"wkq{hp}1"] = \
                (np.ascontiguousarray(a) for a in hilo(cols))
        bk, bq = b[rk], b[rq]
        m["bkq"] = np.ascontiguousarray(
            np.concatenate([bk[:128], bq[:128], bk[128:], bq[128:]]
                           ).reshape(4, P).T * 16.0
        )
        in_maps.append(m)
    return in_maps


def run(inputs, **kwargs):
    nc = _get_nc()
    in_maps = make_in_maps(inputs["x"], inputs["W"], inputs["b"])
    res = run_bass_kernel_spmd(nc, in_maps, core_ids=list(range(8)), **kwargs)
    out = np.empty((N, S, D), dtype=np.float32)
    for c in range(8):
        n, g = divmod(c, 4)
        out[n, :, 256 * g:256 * g + 256] = res.results[c]["o"]
    return out, res


def kernel(**inputs):
    out, _ = run(inputs)
    return out


# revision 66
# speedup vs baseline: 1.0628x; 1.0628x over previous
"""Masked multi-head attention (fused QKV) on 8 trn2 NeuronCores.

Problem (full shapes): x [2, 2048, 1024] f32, W [3072, 1024], b [3072].
  z = x @ W.T + b ; k,q,v = split(z) ; heads H=16, hd=64
  out = softmax(causal(q k^T / sqrt(1024))) v   -> [2, 2048, 1024]

Sharding: core c handles batch n=c//4 and head group g=c%4 (4 heads).
Each core is fully independent (data + head parallel, no collectives).

Per-core device program (v2 of the fp8-DoubleRow kernel):
  0) Projections run as residual-fp8 DoubleRow: x = x_hi + x_lo and
     16*W = W_hi + W_lo in e4m3 (host-side), z = xh@Wh + xh@Wl + xl@Wh
     accumulated in f32 PSUM.  Host packs hi/lo as one [D, 2, *] tensor
     per input so big blocks move in one DMA each.
  1) kq weights are hp-major ([k_hp0|q_hp0|k_hp1|q_hp1]) so the startup
     critical path only loads the hp0 half (256 cols) before the first
     projection; DMA order is cost-model-driven (the 16-engine DMA pool
     serializes transfers at ~360 B/ns and each completion semaphore
     takes ~900ns to propagate).
  2) Scores per (q-block 512, head pair) unit: 2 fp8 DoubleRow matmuls
     per k-tile into a 2-bank PSUM tile, one ACT exp (the global pacer)
     evacuates both heads to bf16 pt.  Unit order
     (0,0),(0,1),(1,0),(2,0),(3,0),(3,1),(2,1),(1,1) starts on the two
     smallest units (whose projections are resident earliest) and ends
     on a mid-size unit whose own PV chains drain history-first.
  3) Filler work (deferred projections, the previous unit's PV chains)
     is placed by a greedy scheduler that simulates ACT/PE prefix times
     with the instruction cost model and only inserts a filler where it
     cannot stall the exp stream; per-filler DMA-readiness estimates
     and deadline slots (for next unit's diag dependencies) constrain
     placement.
  4) PV in natural layout, f32 PSUM [128, 65] per (head, q-tile); DVE
     reciprocal of the fused denominator column + tensor_scalar_mul.
     Late units stream per-q-tile output DMAs so the tail is short.

Numerics identical to v1: bf16 projection ~2e-4; fp8 q/k quantization
dominates at rel_err ~1.3e-2 (harness gate 2e-2).

_split_matmul_waits() is a required legalization for this compiler
build: every engine instruction may carry at most one semaphore wait.
"""

import numpy as np

import concourse.bass as bass
import concourse.mybir as mybir
import concourse.tile as tile
from concourse.bass_utils import run_bass_kernel_spmd

F32 = mybir.dt.float32
BF = mybir.dt.bfloat16
F8 = mybir.dt.float8e4

FP8_SCORES = True

N, S, D = 2, 2048, 1024
H, HD = 16, 64
P = 128
QB = 512                 # q block (free dim per matmul)
NQB = S // QB            # 4
NKT = S // P             # 16 k tiles
ND = D // P              # 8 contraction tiles
NHC = 4                  # heads per core
EKQ = 2 * NHC * HD       # 512 = k+q rows per core
EV = NHC * HD            # 256 = v rows per core
WS = 16.0                # host W/b pre-scale (fp8 range)
SCALE = 1.0 / 32.0 / (WS * WS)   # 1/sqrt(1024), W-scale compensated

AF = mybir.ActivationFunctionType
ALU = mybir.AluOpType

BUILD_LOG = []  # schedule introspection for the offline analyzer

# measured per-(unit, slot) exp end times from the timeline simulator
# (closed-loop calibration of the filler scheduler; see tune.py)
ACT_CAL = {}

# ---- cost-model constants for the greedy filler scheduler ----
ACT_C = 0.8333           # ns/elem on ACT
PE_C = 0.4167            # ns/col full rate
SEM = 160.0              # cross-engine handoff guard
GUARD = 120.0


def _exp_ns(w):
    return 2 * w * ACT_C + 185.0


def _mm_tile_ns(w, direct):
    # two matmuls (one per interleaved head) per k-tile
    if not FP8_SCORES:
        return 2 * w * PE_C
    return (2 * w * PE_C) if direct else (w * PE_C)


def _split_matmul_waits(nc):
    """Move extra semaphore waits onto preceding same-engine NOPs.

    The walrus codegen for self-loading matmuls folds waits into the
    LDWEIGHTS struct, which has room for a single sync-wait command;
    sequencer NOPs on the same engine execute in program order, so
    hoisting each wait onto its own NOP is semantics-preserving.
    """
    import bass_rust

    moved = 0
    for bb in nc.main_func.blocks:
        out = []
        for ins in bb.instructions:
            si = ins.sync_info
            keep = 0 if isinstance(ins, bass_rust.InstMatmult) else 1
            if (
                not isinstance(ins, bass_rust.InstNoOp)
                and si is not None
                and len(si.on_wait) > keep
            ):
                hoist = si.on_wait[keep:] if keep else si.on_wait
                for j, w in enumerate(hoist):
                    out.append(
                        bass_rust.InstNoOp(
                            name=f"{ins.name}-hw{j}",
                            engine=ins.engine,
                            sync_info=mybir.SyncInfo(on_wait=[w], on_update=[]),
                        )
                    )
                    moved += 1
                ins.sync_info = mybir.SyncInfo(
                    on_wait=list(si.on_wait[:keep]), on_update=list(si.on_update)
                )
            out.append(ins)
        bb.instructions[:] = out
    return moved


def build_nc(split_waits=True, act_cal=None):
    if act_cal is None:
        act_cal = ACT_CAL
    nc = bass.Bass()

    # Contraction rows are host-permuted p-major (row r = p*ND + dt) so
    # every DMA slice below keeps >=512B contiguous runs (the DMA cost
    # model halves bandwidth under 512B).  x and wv pack hi/lo planes
    # with u inner; wkq is split per (head pair, plane) so the startup
    # loads only what the first projection needs.
    x2 = nc.dram_tensor("x2", [2 * D, S], F8, kind="ExternalInput")
    wkqt_d = [[nc.dram_tensor(f"wkq{hp}{u}", [D, 2 * P], F8,
                              kind="ExternalInput")
               for u in range(2)] for hp in range(2)]
    wv_d = [nc.dram_tensor(f"wv{u}", [D, EV], F8, kind="ExternalInput")
            for u in range(2)]
    bkq = nc.dram_tensor("bkq", [P, 4], F32, kind="ExternalInput")
    bv = nc.dram_tensor("bv", [1, EV], F32, kind="ExternalInput")
    o = nc.dram_tensor("o", [S, EV], F32, kind="ExternalOutput")

    x_v = x2.rearrange("(p dt u) s -> p dt u s", p=P, u=2)  # [128,8,2,2048]
    wkq_vs = [[wkqt_d[hp][u].rearrange("(p dt) e -> p dt e", p=P)
               for u in range(2)] for hp in range(2)]       # [128,8,256]
    wv_vs = [wv_d[u].rearrange("(p dt) e -> p dt e", p=P)
             for u in range(2)]                             # [128,8,256]
    o_v = o.rearrange("(qt p) c -> p qt c", p=P)            # [128, 16, 256]

    with tile.TileContext(nc) as tc:
        with (
            tc.tile_pool(name="const", bufs=1) as const,
            tc.tile_pool(name="big", bufs=1) as big,
            tc.tile_pool(name="xpool", bufs=2) as xpool,
            tc.tile_pool(name="work", bufs=2) as work,
            tc.tile_pool(name="opool", bufs=2) as opool,
            tc.tile_pool(name="proj_ps", bufs=2, space="PSUM") as proj_ps,
            tc.tile_pool(name="st_ps", bufs=2, space="PSUM") as st_ps,
            tc.tile_pool(name="pv_ps", bufs=2, space="PSUM") as pv_ps,
        ):
            # ---- constants ----
            onesb = const.tile([P, 1], BF)
            nc.vector.memset(onesb, 1.0)
            wsb = const.tile([P, 1], BF)
            nc.vector.memset(wsb, WS)
            # warm the ACT exp table while DMAs run
            dummy = const.tile([1, 2], F32)
            nc.gpsimd.memset(dummy, 0.0)
            nc.scalar.activation(dummy, dummy, AF.Exp)
            # diagonal causal mask for the odd head: mask[p, j] = (j >= p)
            mask_sb = const.tile([P, QB], BF)
            nc.gpsimd.affine_select(
                out=mask_sb,
                in_=onesb.to_broadcast((P, QB)),
                compare_op=ALU.is_ge, fill=0.0,
                base=0, channel_multiplier=-1,
                pattern=[[1, QB]],
            )

            # ---- input DMAs, critical-path first ----
            wkqt = [[const.tile([P, ND, 2 * P], F8, name=f"wkq{hp}{u}")
                     for u in range(2)] for hp in range(2)]
            wvt = [const.tile([P, ND, EV], F8, name=f"wv{u}")
                   for u in range(2)]
            bkq_sb = const.tile([P, 4], F32)
            bvb = const.tile([P, EV], F32)
            xqbs = []
            for qb in range(NQB):
                xq = xpool.tile([P, ND, 2, QB], F8, tag=f"xqb{qb}", bufs=1,
                                name=f"xqb{qb}")
                xqbs.append(xq)

            # DMA cost model: gens serialize on HWDGE (625ns each),
            # transfers serialize on the 16-engine pool at ~360 B/ns,
            # completion semaphores take ~950ns to reach consumers.
            st_dma = {"gen": 1.7e3, "tx": 0.0}

            def dma(out_ap, in_ap, kbytes):
                nc.sync.dma_start(out_ap, in_ap)
                st_dma["gen"] += 625.0
                start = max(st_dma["gen"], st_dma["tx"])
                st_dma["tx"] = start + kbytes * 1024.0 / 360.0
                return st_dma["tx"] + 950.0

            r_bkq = dma(bkq_sb, bkq[:, :], 2)
            r_wkq00 = dma(wkqt[0][0], wkq_vs[0][0], 256)
            r_xq0h = dma(xqbs[0][:, :, 0], x_v[:, :, 0, 0:QB], 512)
            r_xq0l = dma(xqbs[0][:, :, 1], x_v[:, :, 1, 0:QB], 512)
            r_wkq01 = dma(wkqt[0][1], wkq_vs[0][1], 256)
            r_xq1h = dma(xqbs[1][:, :, 0], x_v[:, :, 0, QB:2 * QB], 512)
            r_xq1l = dma(xqbs[1][:, :, 1], x_v[:, :, 1, QB:2 * QB], 512)
            r_wvh = dma(wvt[0], wv_vs[0], 256)
            r_wvl = dma(wvt[1], wv_vs[1], 256)
            r_bv = dma(bvb, bv[:, :].partition_broadcast(P), 1)
            r_wv = max(r_wvl, r_bv)

            # ---- persistent state ----
            # zkq [p, kq, hp, s]; e-tile t -> (kq=t%2, hp=t//2)
            if FP8_SCORES:
                zkq = big.tile([P, 2, 2, S], F8)
                # DoubleRow re-layout [hl*32+p, kq, hp, g, s], hd = g*32+p
                kq8 = big.tile([HD, 2, 2, 2, S], F8)
            else:
                zkq = big.tile([P, 4, S], BF)
            # v natural + WS column: [p, ktile, head, 65]
            vsb = big.tile([P, NKT, NHC, HD + 1], BF)
            nc.vector.tensor_copy(
                vsb[:, :, :, HD:HD + 1],
                wsb[:, :, None].to_broadcast((P, NKT, NHC, 1)),
            )
            # exp(S^T) per unit: [p, head, ktile, q], triple-buffered so
            # a unit's PV chains may drain up to two units later
            pts = [big.tile([P, 2, NKT, QB], BF, name=f"pt{i}")
                   for i in range(3)]

            proj_state = {}
            DR = mybir.MatmulPerfMode.DoubleRow
            # residual passes: z = xh@Wh + xh@Wl + xl@Wh, in an order
            # chosen per granule so the last pass waits on the DMA that
            # lands last for that block.
            PASS_WL = ((0, 0), (1, 0), (0, 1))   # W-lo last (qb0 blocks)
            PASS_XL = ((0, 0), (0, 1), (1, 0))   # x-lo last (qb>=1)

            def proj_kq_half(qb, t, half, passes=PASS_XL, evac_act=False):
                if half == 0:
                    pzp = proj_ps.tile([P, QB], F32, tag="projps",
                                       name=f"pzp{qb}_{t}")
                    proj_state[(qb, t)] = pzp
                else:
                    pzp = proj_state.pop((qb, t)) if half == 2 \
                        else proj_state[(qb, t)]
                xi, wi = passes[half]
                for p2 in range(4):
                    nc.tensor.matmul(
                        pzp,
                        lhsT=wkqt[t // 2][wi][:, 2 * p2:2 * p2 + 2,
                                              (t % 2) * P:(t % 2 + 1) * P],
                        rhs=xqbs[qb][:, 2 * p2:2 * p2 + 2, xi, :],
                        start=(half == 0 and p2 == 0),
                        stop=(half == 2 and p2 == 3),
                        perf_mode=DR,
                    )
                if half < 2:
                    return
                if FP8_SCORES:
                    out = zkq[:, t % 2, t // 2, qb * QB:(qb + 1) * QB]
                else:
                    out = zkq[:, t, qb * QB:(qb + 1) * QB]
                if evac_act:
                    nc.scalar.activation(
                        out, pzp, AF.Identity, bias=bkq_sb[:, t:t + 1]
                    )
                else:
                    nc.vector.tensor_scalar_add(out, pzp, bkq_sb[:, t:t + 1])
                if FP8_SCORES:
                    # per-plane DoubleRow re-layout via SBUF->SBUF DMAs,
                    # skipping planes no history matmul ever reads:
                    # k-planes of qb3 (qb3 k-tiles are always diagonal),
                    # q-planes of qb0 (no history in qb0 units), qb1-hp0
                    # (unit (1,0) is all-direct)
                    kqp, hp = t % 2, t // 2
                    need = ((kqp == 0 and qb <= 2)
                            or (kqp == 1 and (qb >= 2
                                              or (qb == 1 and hp == 1))))
                    qw = slice(qb * QB, (qb + 1) * QB)
                    if need:
                        for hl in range(2):
                            for g in range(2):
                                nc.sync.dma_start(
                                    kq8[hl * 32:(hl + 1) * 32, kqp, hp,
                                        g, qw],
                                    zkq[hl * HD + g * 32:
                                        hl * HD + g * 32 + 32,
                                        kqp, hp, qw],
                                )

            def proj_v1(qb, qt4):
                qt = qb * 4 + qt4
                pvp = proj_ps.tile([P, QB], F32, tag="projps",
                                   name=f"pvp{qt}")
                for hi in range(3):
                    xi, wi = PASS_XL[hi]
                    for p2 in range(4):
                        nc.tensor.matmul(
                            pvp[:, :EV],
                            lhsT=xqbs[qb][:, 2 * p2:2 * p2 + 2, xi,
                                          qt4 * P:(qt4 + 1) * P],
                            rhs=wvt[wi][:, 2 * p2:2 * p2 + 2, :],
                            start=(hi == 0 and p2 == 0),
                            stop=(hi == 2 and p2 == 3),
                            perf_mode=DR,
                        )
                nc.vector.tensor_tensor(
                    vsb[:, qt, :, 0:HD],
                    pvp[:, :EV].rearrange("p (h d) -> p h d", d=HD),
                    bvb.rearrange("p (h d) -> p h d", d=HD),
                    mybir.AluOpType.add,
                )

            def pv_chunk(qb, hp, qt4, hl, osb, pt, hist_first=False,
                         dma_qt=False):
                nkt_q = 4 * qb + qt4 + 1
                if hist_first:
                    kts = list(range(nkt_q))
                else:
                    kts = list(range(4 * qb, nkt_q)) + list(range(0, 4 * qb))
                pvo = pv_ps.tile([P, HD + 1], F32, tag="pv", name="pvo")
                for i, kt in enumerate(kts):
                    nc.tensor.matmul(
                        pvo,
                        lhsT=pt[:, hl, kt, qt4 * P:(qt4 + 1) * P],
                        rhs=vsb[:, kt, 2 * hp + hl, :],
                        start=(i == 0), stop=(i == nkt_q - 1),
                    )
                h = 2 * hp + hl
                rs = work.tile([P, 1], F32, tag="rs", bufs=4)
                nc.vector.reciprocal(rs, pvo[:, HD:HD + 1])
                nc.vector.tensor_scalar_mul(
                    osb[:, qt4, HD * h:HD * (h + 1)],
                    pvo[:, 0:HD], rs,
                )
                if dma_qt and hl == 1:
                    nc.sync.dma_start(
                        o_v[:, qb * 4 + qt4, hp * P:(hp + 1) * P],
                        osb[:, qt4, hp * P:(hp + 1) * P],
                    )

            # ---- greedy filler scheduler state ----
            # Fillers live in global FIFO queues and may spill across
            # unit boundaries; (ui, slot) deadlines encode the real
            # consumers (pt-ring WAR at slot 4 of unit ui+2, kq8
            # relayout latency before a diag reader, etc).
            sched = {"pe": 0.0, "act": 0.0}
            queues = []
            exp_tail = []
            vsb_cnt = [0]  # vsb k-tiles written so far (in qt order)

            def attn_scores(ui, qb, hp, pt, hist_first=False):
                if hist_first:
                    kts = list(range(0, 4 * qb + 4))
                else:
                    kts = list(range(4 * qb, 4 * qb + 4)) + \
                        list(range(4 * qb))

                def emit(f, forced=False):
                    BUILD_LOG.append(("fill", f.get("tag", "?"),
                                      f.get("ready", 0.0), f["pe"], forced))
                    f["fn"]()
                    vsb_cnt[0] += f.get("vw", 0)
                    sched["pe"] = max(sched["pe"], f.get("ready", 0.0)) \
                        + f["pe"]

                def v_ok(f):
                    # PV chunks may not precede the V tiles they read
                    return f.get("vneed", 0) <= vsb_cnt[0]

                def force_v(n):
                    # emit pending fillers from the earliest queues until
                    # n vsb tiles are written (queue order keeps every
                    # intermediate pop legal)
                    while vsb_cnt[0] < n:
                        for g in queues:
                            if g and v_ok(g[0]):
                                emit(g.pop(0), forced=True)
                                break
                        else:
                            raise RuntimeError("vsb writer missing")

                for i, kt in enumerate(kts):
                    pos = (ui, i)
                    r = kt - 4 * qb
                    diag = 0 <= r < 4
                    off = P * r if diag else 0
                    w = QB - off
                    # the first unit's history and every unit's first
                    # three history tiles read zkq directly: the own-
                    # block kq8 relayout (evac -> SBUF DMA -> sem, ~3us)
                    # would otherwise stall the exp stream after the diag
                    direct = diag or ui == 1 or \
                        (not hist_first and 4 <= i < 7)
                    mmc = _mm_tile_ns(w, direct)
                    # forced fillers (deadline reached)
                    for g in queues:
                        while g and g[0].get("dl") and g[0]["dl"] <= pos:
                            if not v_ok(g[0]):
                                force_v(g[0]["vneed"])
                            emit(g.pop(0), forced=True)
                    # opportunistic fillers while ACT has slack
                    progress = True
                    while progress:
                        progress = False
                        for g in queues:
                            if (g and g[0].get("es", (-1, -1)) <= pos
                                    and v_ok(g[0])
                                    and g[0].get("ready", 0.0) <= sched["pe"]
                                    and sched["pe"] + g[0]["pe"] + mmc
                                    + GUARD <= sched["act"]):
                                emit(g.pop(0))
                                progress = True
                    # st ring WAR: mm_i waits exp_{i-2}
                    if len(exp_tail) >= 2:
                        sched["pe"] = max(sched["pe"],
                                          exp_tail[-2] + 100.0)
                    stp = st_ps.tile([P, 2, QB], F32, tag="st")
                    for hl in range(2):
                        if FP8_SCORES and direct:
                            base = HD * hl
                            nc.tensor.matmul(
                                stp[:, hl, off:QB],
                                lhsT=zkq[base:base + HD, 0, hp,
                                         kt * P:(kt + 1) * P],
                                rhs=zkq[base:base + HD, 1, hp,
                                        qb * QB + off:(qb + 1) * QB],
                                start=True, stop=True,
                            )
                        elif FP8_SCORES:
                            nc.tensor.matmul(
                                stp[:, hl, off:QB],
                                lhsT=kq8[hl * 32:(hl + 1) * 32, 0, hp, :,
                                         kt * P:(kt + 1) * P],
                                rhs=kq8[hl * 32:(hl + 1) * 32, 1, hp, :,
                                        qb * QB + off:(qb + 1) * QB],
                                start=True, stop=True,
                                perf_mode=DR,
                            )
                        else:
                            base = HD * hl
                            nc.tensor.matmul(
                                stp[:, hl, off:QB],
                                lhsT=zkq[base:base + HD, 2 * hp,
                                         kt * P:(kt + 1) * P],
                                rhs=zkq[base:base + HD, 2 * hp + 1,
                                        qb * QB + off:(qb + 1) * QB],
                                start=True, stop=True,
                            )
                    sched["pe"] += mmc
                    est = max(sched["act"], sched["pe"] + SEM)
                    BUILD_LOG.append(("exp", ui, i, w,
                                      sched["act"], sched["pe"]))
                    nc.scalar.activation(
                        pt[:, :, kt, off:QB],
                        stp[:, :, off:QB],
                        AF.Exp, scale=SCALE,
                    )
                    sched["act"] = act_cal.get((ui, i), est + _exp_ns(w)) \
                        if act_cal else est + _exp_ns(w)
                    exp_tail.append(sched["act"])
                    del exp_tail[:-2]
                    if diag:
                        nc.gpsimd.affine_select(
                            out=pt[:, 0, kt, off:QB],
                            in_=pt[:, 0, kt, off:QB],
                            compare_op=ALU.is_ge, fill=0.0,
                            base=0, channel_multiplier=-1,
                            pattern=[[1, w]],
                        )
                        nc.vector.tensor_mul(
                            out=pt[:, 1, kt, off:QB],
                            in0=pt[:, 1, kt, off:QB],
                            in1=mask_sb[:, 0:w],
                        )

            def drain_queues():
                for g in queues:
                    while g:
                        f = g.pop(0)
                        BUILD_LOG.append(("fill", f.get("tag", "?"),
                                          f.get("ready", 0.0), f["pe"],
                                          "drain"))
                        f["fn"]()
                        vsb_cnt[0] += f.get("vw", 0)

            # ---- filler group constructors ----
            def K(qb2, pair, dl=None, tiles=None):
                # one ATOMIC filler per e-tile granule (alloc..evac in a
                # single pop) so the 2-deep proj PSUM ring can never see
                # three live accumulations regardless of queue order
                passes = PASS_WL if qb2 == 0 else PASS_XL
                if tiles is None:
                    tiles = (2 * pair, 2 * pair + 1)
                ready = max(max(x_ready[qb2]),
                            max(wkq_ready[pair]))

                def gran(t, q, ps):
                    for h in range(3):
                        proj_kq_half(q, t, h, passes=ps)

                out = []
                for t in tiles:
                    f = {
                        "fn": (lambda t=t, q=qb2, ps=passes:
                               gran(t, q, ps)),
                        "tag": f"K{qb2}t{t}",
                        "pe": 1380.0, "ready": ready,
                    }
                    if dl is not None:
                        f["dl"] = dl
                    out.append(f)
                return out

            def V(qb2):
                ready = max(x_ready[qb2][1], r_wv, r_bv)
                return [
                    {"fn": (lambda q4=q4, q=qb2: proj_v1(q, q4)),
                     "tag": f"V{qb2}q{q4}", "vw": 1,
                     "pe": 840.0, "ready": ready}
                    for q4 in range(4)
                ]

            def PV(pqb, php, ppt, v_qb=None, v_pre=None, dma_qt=False,
                   dl=None):
                # previous unit's PV chains; V-projections whose vsb
                # tiles those chains read go before (v_pre, whole block)
                # or interleaved per q-tile (v_qb)
                out = V(v_pre) if v_pre is not None else []
                vg = V(v_qb) if v_qb is not None else []
                for q4 in range(4):
                    if vg:
                        out.append(vg[q4])
                    for hl in range(2):
                        out.append({
                            "fn": (lambda q4=q4, hl=hl:
                                   pv_chunk(pqb, php, q4, hl, osbs[pqb],
                                            ppt, dma_qt=dma_qt)),
                            "tag": f"PV{pqb}{php}q{q4}l{hl}",
                            "vneed": 4 * pqb + q4 + 1,
                            "pe": (4 * pqb + q4 + 1) * 27.0 + 250.0,
                        })
                if not dma_qt:
                    # half-row block DMA rides at the queue tail so it
                    # always follows the last chunk that writes osb
                    out.append({
                        "fn": (lambda: nc.sync.dma_start(
                            o_v[:, pqb * 4:(pqb + 1) * 4,
                                php * P:(php + 1) * P],
                            osbs[pqb][:, :, php * P:(php + 1) * P])),
                        "tag": f"ODMA{pqb}{php}", "pe": 60.0,
                    })
                if dl is not None:
                    for f in out:
                        f["dl"] = dl
                return out

            # ---- prologue: pass-major proj of (k,q) head pair 0, qb0 ----
            for half in range(3):
                for t in (0, 1):
                    proj_kq_half(0, t, half, passes=PASS_WL,
                                 evac_act=(t == 1))
            # late-needed inputs issued after the prologue so its kq8
            # relayouts aren't head-of-line-blocked on the FIFO DMA pool
            st_dma["gen"] += 2 * 625.0
            r_xq2 = dma(xqbs[2][:, :, :], x_v[:, :, :, 2 * QB:3 * QB], 1024)

            def late_inputs():
                # issued from inside unit 2 (after K(2,0)'s relayouts
                # enter the FIFO queues) -- see the (2,1)-slot filler
                nc.sync.dma_start(wkqt[1][0], wkq_vs[1][0])
                nc.sync.dma_start(wkqt[1][1], wkq_vs[1][1])
                nc.sync.dma_start(xqbs[3][:, :, :],
                                  x_v[:, :, :, 3 * QB:4 * QB])

            r_wkq10, r_wkq11, r_xq3 = 24.2e3, 25.0e3, 27.9e3
            x_ready = [(r_xq0h, r_xq0l), (r_xq1h, r_xq1l),
                       (r_xq2, r_xq2), (r_xq3, r_xq3)]
            wkq_ready = [(r_wkq00, r_wkq01), (r_wkq10, r_wkq11)]

            # ---- unit schedule ----
            # Big units early: their exp windows host the projection
            # fillers later units depend on; (0,1) last keeps the tail
            # tiny (its own PV chains are 1-4 matmuls each).
            units = [(0, 0), (1, 0), (2, 0), (3, 0),
                     (3, 1), (2, 1), (1, 1), (0, 1)]
            # deadline for PV-of-(ui-1), keyed by current ui: slot 4 of
            # unit ui+2 (pt ring of 3; WAR with its history exps), or
            # slot 0 where that writer's diag already overlaps.
            PV_DL = {1: (3, 4), 2: (4, 4), 3: (5, 0),
                     4: (6, 0), 5: (7, 0), 6: None, 7: None}
            # K-group deadlines: each unit's diag (and first-3-history)
            # matmuls consume zkq directly before its slot 0/4; kq8
            # relayouts fire per-plane at the evacuations.
            K_PLAN = {
                0: [(1, 0, None, (1, 0))],
                1: [(2, 0, None, (2, 0))],
                2: [(3, 0, None, (3, 0)), (0, 1, None, (3, 12))],
                3: [(3, 1, None, (3, 10)), (1, 1, None, (4, 2))],
                4: [(2, 1, None, (4, 6))],
            }
            osbs = {}
            prev = None
            sched["pe"] = 8100.0
            sched["act"] = 8300.0
            for ui, (qb, hp) in enumerate(units):
                pt = pts[ui % 3]
                last = ui == len(units) - 1
                if qb not in osbs:
                    osbs[qb] = opool.tile([P, 4, EV], F32, tag="osb",
                                          bufs=4, name=f"osb{qb}")
                if prev is not None:
                    pqb, php, ppt = prev
                    # V(j) interleaves with the first PV chains reading it
                    v_qb = {1: 0, 2: 1, 3: 2, 4: 3}.get(ui)
                    queues.append(PV(pqb, php, ppt, v_qb=v_qb,
                                     dma_qt=(ui == 7), dl=PV_DL[ui]))
                for kqb, kpair, ktiles, kdl in K_PLAN.get(ui, []):
                    queues.append(K(kqb, kpair, dl=kdl, tiles=ktiles))
                if ui == 2:
                    queues.append([{"fn": late_inputs, "tag": "lateDMA",
                                    "pe": 100.0, "es": (2, 1),
                                    "dl": (2, 2)}])
                if last:
                    own = []
                    for q4 in range(4):
                        for hl in range(2):
                            own.append({
                                "fn": (lambda q4=q4, hl=hl:
                                       pv_chunk(qb, hp, q4, hl, osbs[qb],
                                                pt, hist_first=True,
                                                dma_qt=True)),
                                "tag": f"OWNq{q4}l{hl}",
                                "vneed": 4 * qb + q4 + 1,
                                "pe": (4 * qb + q4 + 1) * 27.0 + 250.0,
                                "es": (ui, 4 * qb + q4 + 1),
                            })
                    queues.append(own)
                attn_scores(ui, qb, hp, pt, hist_first=last)
                prev = (qb, hp, pt)
            drain_queues()

    if split_waits:
        _split_matmul_waits(nc)
    return nc


_nc_cache = None


def _get_nc():
    global _nc_cache
    if _nc_cache is None:
        _nc_cache = build_nc()
    return _nc_cache


def make_in_maps(x, W, b):
    import ml_dtypes

    f8 = ml_dtypes.float8_e4m3
    # p-major row permutation of the contraction dim: row r = p*ND + dt
    # picks original row dt*128 + p (x and W share it, so z is identical)
    perm = (np.arange(D).reshape(ND, P).T).reshape(-1)

    def hilo(a):
        hi = a.astype(f8)
        lo = (a - hi.astype(np.float32)).astype(f8)
        return hi, lo

    def hilo_pk(a):
        # rows (p dt) -> interleave planes u inner: rows (p dt u)
        hi, lo = hilo(a)
        return np.ascontiguousarray(
            np.stack([hi, lo], axis=1).reshape(2 * a.shape[0], a.shape[1])
        )

    x = np.asarray(x, dtype=np.float32)
    W = np.asarray(W, dtype=np.float32)
    b = np.asarray(b, dtype=np.float32)
    in_maps = []
    x2s = [hilo_pk(x[n].T[perm]) for n in range(N)]
    for c in range(8):
        n, g = divmod(c, 4)
        rk = slice(256 * g, 256 * g + 256)
        rq = slice(D + 256 * g, D + 256 * g + 256)
        rv = slice(2 * D + 256 * g, 2 * D + 256 * g + 256)
        Wk, Wq, Wv = W[rk], W[rq], W[rv]
        wvh, wvl = hilo(Wv.T[perm] * 16.0)
        m = {"x2": x2s[n],
             "wv0": np.ascontiguousarray(wvh),
             "wv1": np.ascontiguousarray(wvl),
             "bv": np.ascontiguousarray(b[rv].reshape(1, EV) * 16.0)}
        for hp in range(2):
            hr = slice(128 * hp, 128 * hp + 128)
            cols = np.concatenate([Wk[hr], Wq[hr]], axis=0).T[perm] * 16.0
            m[f"wkq{hp}0"], m[f"wkq{hp}1"] = \
                (np.ascontiguousarray(a) for a in hilo(cols))
        bk, bq = b[rk], b[rq]
        m["bkq"] = np.ascontiguousarray(
            np.concatenate([bk[:128], bq[:128], bk[128:], bq[128:]]
                           ).reshape(4, P).T * 16.0
        )
        in_maps.append(m)
    return in_maps


def run(inputs, **kwargs):
    nc = _get_nc()
    in_maps = make_in_maps(inputs["x"], inputs["W"], inputs["b"])
    res = run_bass_kernel_spmd(nc, in_maps, core_ids=list(range(8)), **kwargs)
    out = np.empty((N, S, D), dtype=np.float32)
    for c in range(8):
        n, g = divmod(c, 4)
        out[n, :, 256 * g:256 * g + 256] = res.results[c]["o"]
    return out, res


def kernel(**inputs):
    out, _ = run(inputs)
    return out
